# revision 19
# baseline (speedup 1.0000x reference)
"""SSIM3D loss kernel for 8 Trainium2 NeuronCores (v2).

Strategy (hardcoded for inputs [2, 3, 16, 256, 256] fp32):
  - Shard across 8 cores as (batch 2) x (H quarter 4). Each core: C=3,
    T=16, 64 output H rows (+3-row halos), W=256.
  - 4 conv fields: a=x+y, b=x-y, m=2xy, s=x^2+y^2 (all zero in padded
    regions, matching the reference's zero-padded 'same' conv). With
    A1=conv(a), B1=conv(b), D=conv(m), S=conv(s):
      u = (A1^2-B1^2)/2 = 2*mu1*mu2      v = (A1^2+B1^2)/2 = mu1^2+mu2^2
      num = (u+C1)*((D+C2)-u)            den = (v+C1)*((S+C2)-v)
      ssim = num/den
  - Pass A (PE, data-as-lhsT): fused H+T 7-tap conv as banded matmuls,
    partitions packed (h_sub=8, t=16); output transposed to [w, ht].
    H halos via two-matmul PSUM accumulation (wa from j=k, wb from j=k+1).
  - Bridge pa->SBUF bf16 split: DVE copies bank0 half, ACT copies bank1.
  - Pass B (PE, weights-stationary): W 7-tap conv per 128-col w chunk,
    one N=512 matmul per chunk; chunk-boundary taps dropped with
    renormalized truncated windows (golden-sim validated, ~2.7e-4).
  - Staging: ACT Square(sqrt(.5)*x) writes aa/bb straight from PSUM;
    DVE copies D/S from PSUM. Both land in one fp16 stage tile per c.
  - Pointwise chain per half-channel (FD=1024) with fused DVE ops:
    u,v (TT), P/num/Q/den (scalar_tensor_tensor), custom fast reciprocal,
    and tensor_tensor_reduce for the final multiply + partition reduction.
    Chain ops of channel c interleave into channel c+1's k-loop.
  - Host sums the per-core accumulators: loss = 1 - total/N.
"""
import os
import numpy as np
import ml_dtypes

BF16 = ml_dtypes.bfloat16
F16 = np.float16

B, C, T, H, W = 2, 3, 16, 256, 256
WS, SIGMA, PAD = 7, 1.5, 3
C1, C2 = np.float32(1e-4), np.float32(9e-4)
NCORES = 8
HQ = H // 4          # 64 output rows per core
NJ = 9               # input h tiles of 8 rows covering [-3, 69)
NK = 8               # output h tiles of 8 rows covering [0, 64)
FREE = NJ * W        # 2304
NACC = 6             # 3 channels x 2 half-channel groups

last_exec_time_ns = None
last_results = None


def _comp_round(weights):
    """Round to bf16 greedily (largest magnitude first), keeping the
    cumulative rounding error near zero."""
    w = np.asarray(weights, dtype=np.float64).ravel()

    def neighbors(v):
        b = np.float64(np.float32(v).astype(BF16).astype(np.float32))
        cands = {b}
        u = int(np.array(b, dtype=BF16).view(np.uint16))
        for d in (-1, 1):
            cands.add(np.float64(np.uint16((u + d) & 0xFFFF).view(BF16).astype(np.float32)))
        return cands

    order = np.argsort(-np.abs(w))
    out = np.empty_like(w)
    errsum = 0.0
    for i in order:
        best = min(neighbors(w[i]), key=lambda cnd: abs(errsum + (cnd - w[i])))
        out[i] = best
        errsum += best - w[i]
    return out.reshape(np.shape(weights)).astype(np.float32)


def _gaussian():
    coords = np.arange(WS, dtype=np.float64) - PAD
    g = np.exp(-(coords ** 2) / (2.0 * SIGMA ** 2))
    return g / g.sum()


def _build_weights():
    """wa, wb: banded fused H+T conv [128,128].
    W00, W11: per-chunk 1-D W conv [128,128] with renormalized truncated
    windows at the chunk boundary (image edges keep zero-pad truncation)."""
    g = _gaussian()
    wht = _comp_round(np.outer(g, g))

    wa = np.zeros((128, 128), np.float32)
    wb = np.zeros((128, 128), np.float32)
    for i in range(8):
        for o in range(8):
            dh = i - o - 3
            if -3 <= dh <= 3:
                for ti in range(16):
                    for to in range(16):
                        dt_ = ti - to
                        if -3 <= dt_ <= 3:
                            wa[i * 16 + ti, o * 16 + to] = wht[dh + 3, dt_ + 3]
            dh = i + 5 - o
            if -3 <= dh <= 3:
                for ti in range(16):
                    for to in range(16):
                        dt_ = ti - to
                        if -3 <= dt_ <= 3:
                            wb[i * 16 + ti, o * 16 + to] = wht[dh + 3, dt_ + 3]

    gw = _comp_round(g).astype(np.float64)
    Wm = [np.zeros((128, 128), np.float32) for _ in range(2)]
    for m in range(2):
        base = m * 128
        for o in range(128):
            og = base + o
            true_taps = [d for d in range(-3, 4) if 0 <= og + d < W]
            pres = [d for d in true_taps if 0 <= o + d < 128]
            scale = sum(gw[d + 3] for d in true_taps) / sum(gw[d + 3] for d in pres)
            for d in pres:
                Wm[m][o + d, o] = np.float32(gw[d + 3] * scale)
    return (wa.astype(BF16), wb.astype(BF16),
            Wm[0].astype(BF16), Wm[1].astype(BF16))


def _build_slab(x_bf, b, q):
    """Per-core input slab [3, 128, 2304] bf16: partition = h_sub*16+t,
    free = j*256+w; local h = 8j - 3 + h_sub relative to row 64q."""
    pad = np.zeros((C, T, NJ * 8, W), dtype=BF16)
    lo, hi = HQ * q - 3, HQ * q + 69
    s_lo, s_hi = max(0, lo), min(H, hi)
    pad[:, :, (s_lo - lo):(s_hi - lo), :] = x_bf[b, :, :, s_lo:s_hi, :]
    arr = pad.reshape(C, T, NJ, 8, W).transpose(0, 3, 1, 2, 4)
    return np.ascontiguousarray(arr.reshape(C, 128, FREE))


def _build_program():
    import concourse.bass as bass
    import concourse.mybir as mybir
    from concourse import bacc, tile
    from concourse.dve_ops import RECIP_APPROX_FAST_CONSTS, RECIPROCAL_APPROX_FAST
    from contextlib import ExitStack

    dt = mybir.dt
    Alu = mybir.AluOpType
    Act = mybir.ActivationFunctionType
    SQ5 = float(np.sqrt(0.5))
    RCST = RECIP_APPROX_FAST_CONSTS

    nc = bacc.Bacc()
    fin = [nc.dram_tensor(nm, [C, 128, FREE], dt.bfloat16, kind="ExternalInput")
           for nm in ("fa", "fb", "fm", "fs")]
    wdr = nc.dram_tensor("wcat", [128, 512], dt.bfloat16, kind="ExternalInput")
    osum = nc.dram_tensor("osum", [128, NACC], dt.float32, kind="ExternalOutput")

    with tile.TileContext(nc) as tc, ExitStack() as ctx:
        wpool = ctx.enter_context(tc.tile_pool(name="w", bufs=1))
        fpool = ctx.enter_context(tc.tile_pool(name="f", bufs=3))
        vpool = ctx.enter_context(tc.tile_pool(name="v", bufs=3))
        spool = ctx.enter_context(tc.tile_pool(name="st", bufs=2))
        ppool = ctx.enter_context(tc.tile_pool(name="pt", bufs=2))
        psA = ctx.enter_context(tc.tile_pool(name="psA", bufs=2, space="PSUM"))
        psB = ctx.enter_context(tc.tile_pool(name="psB", bufs=2, space="PSUM"))

        # weights: one DMA into staging, one DVE copy bridge so matmuls wait
        # on a single engine semaphore instead of DMA-queue semaphores
        wstg = wpool.tile([128, 512], dt.bfloat16, name="wsg", tag="wsg")
        nc.sync.dma_start(wstg[:], wdr[:])
        wcat = wpool.tile([128, 512], dt.bfloat16)
        nc.vector.tensor_copy(wcat[:], wstg[:])
        wa = wcat[:, 0:128]
        wb = wcat[:, 128:256]
        w00 = wcat[:, 256:384]
        w11 = wcat[:, 384:512]

        sums = wpool.tile([128, NACC], dt.float32)

        HALF = 5 * 256  # j tiles 0..4 cover pass A for k tiles 0..3
        fields_by_c = []
        for c in range(C):
            ftiles = []
            for i, nm in enumerate(("a", "b", "m", "s")):
                ft = fpool.tile([128, FREE], dt.bfloat16, tag=nm)
                ftiles.append(ft)
            for i in range(4):
                nc.sync.dma_start(ftiles[i][:, 0:HALF], fin[i][c][:, 0:HALF])
            for i in range(4):
                nc.sync.dma_start(ftiles[i][:, HALF:FREE], fin[i][c][:, HALF:FREE])
            fields_by_c.append(tuple(ftiles))

        def pass_a(c, k):
            """8 MMs -> pa [128, 2, 4, 128] (wc, fi, ht)."""
            pa = psA.tile([128, 2, 4, 128], dt.float32, tag="pa")
            fields = fields_by_c[c]
            for wc in range(2):
                for fi in range(4):
                    j0 = k * 256 + wc * 128
                    j1 = (k + 1) * 256 + wc * 128
                    f = fields[fi]
                    nc.tensor.matmul(pa[:, wc, fi], f[:, j0:j0 + 128],
                                     wa, start=True, stop=False)
                    nc.tensor.matmul(pa[:, wc, fi], f[:, j1:j1 + 128],
                                     wb, start=False, stop=True)
            return pa

        NO_TTR = bool(int(os.environ.get("SSIM_NO_TTR", "1")))
        STT_ACCUM = bool(int(os.environ.get("SSIM_STT_ACCUM", "1")))
        NO_STT = bool(int(os.environ.get("SSIM_NO_STT", "0")))
        NO_ACTCOPY = bool(int(os.environ.get("SSIM_NO_ACTCOPY", "0")))
        RANK4 = bool(int(os.environ.get("SSIM_RANK4", "0")))
        NO_GPSIMD = bool(int(os.environ.get("SSIM_NO_GPSIMD", "0")))

        def bridge(pa):
            """pa PSUM -> v SBUF bf16; DVE takes bank pair 0, ACT bank pair 1."""
            v = vpool.tile([128, 2, 4, 128], dt.bfloat16, tag="v")
            nc.vector.tensor_copy(v[:, 0], pa[:, 0])
            if NO_ACTCOPY:
                nc.vector.tensor_copy(v[:, 1], pa[:, 1])
            else:
                nc.scalar.copy(v[:, 1], pa[:, 1])
            return v

        def pass_b(v):
            """2 N=512 MMs -> pb [128, 2, 4, 128] (m, fi, ht), partition=w'."""
            pb = psB.tile([128, 2, 4, 128], dt.float32, tag="pb")
            nc.tensor.matmul(pb[:, 0], w00, v[:, 0], start=True, stop=True)
            nc.tensor.matmul(pb[:, 1], w11, v[:, 1], start=True, stop=True)
            return pb

        def stage(st, k, pb):
            """aa/bb via ACT Square from PSUM; D/S via DVE copy from PSUM.
            st layout is quantity-major [128, q, k, m, ht] so chain views
            are dense rank-2."""
            if RANK4:
                nc.scalar.activation(st[:, 0:2, k, :, :], pb[:, :, 0:2, :],
                                     Act.Square, scale=SQ5)
                nc.vector.tensor_copy(st[:, 2:4, k, :, :], pb[:, :, 2:4, :])
            else:
                for m in range(2):
                    nc.scalar.activation(st[:, 0:2, k, m, :], pb[:, m, 0:2, :],
                                         Act.Square, scale=SQ5)
                    nc.vector.tensor_copy(st[:, 2:4, k, m, :], pb[:, m, 2:4, :])

        def chain_ops(st, c, g):
            """8 lambdas: pointwise for half-channel group g of channel c."""
            ops = []
            if True:
                k0, k1 = g * 4, g * 4 + 4
                mrg = lambda q: st[:, q, k0:k1, :, :].rearrange(
                    "p a b n -> p (a b n)")
                aa, bb, Dv, Sv = mrg(0), mrg(1), mrg(2), mrg(3)
                shape = [128, 1024]
                u = ppool.tile(shape, dt.float16, tag="u")
                vv = ppool.tile(shape, dt.float16, tag="vv")
                P = ppool.tile(shape, dt.float16, tag="P")
                num = ppool.tile(shape, dt.float16, tag="num")
                Q = ppool.tile(shape, dt.float16, tag="Q")
                den = ppool.tile(shape, dt.float32, tag="den")
                rec = ppool.tile(shape, dt.float16, tag="rec")
                sout = ppool.tile(shape, dt.float16, tag="sout")
                slot = c * 2 + g
                teng = nc.vector if NO_GPSIMD else nc.gpsimd
                ops.append(lambda u=u, aa=aa, bb=bb, teng=teng:
                           teng.tensor_sub(u[:], aa, bb))
                ops.append(lambda vv=vv, aa=aa, bb=bb, teng=teng:
                           teng.tensor_add(vv[:], aa, bb))
                if NO_STT:
                    t0 = ppool.tile(shape, dt.float16, tag="t0")
                    t1 = ppool.tile(shape, dt.float16, tag="t1")
                    ops.append(lambda t0=t0, Dv=Dv:
                               nc.vector.tensor_scalar_add(t0[:], Dv, float(C2)))
                    ops.append(lambda P=P, t0=t0, u=u:
                               nc.vector.tensor_sub(P[:], t0[:], u[:]))
                    ops.append(lambda t1=t1, u=u:
                               nc.vector.tensor_scalar_add(t1[:], u[:], float(C1)))
                    ops.append(lambda num=num, t1=t1, P=P:
                               nc.vector.tensor_mul(num[:], t1[:], P[:]))
                    t2 = ppool.tile(shape, dt.float16, tag="t2")
                    t3 = ppool.tile(shape, dt.float16, tag="t3")
                    ops.append(lambda t2=t2, Sv=Sv:
                               nc.vector.tensor_scalar_add(t2[:], Sv, float(C2)))
                    ops.append(lambda Q=Q, t2=t2, vv=vv:
                               nc.vector.tensor_sub(Q[:], t2[:], vv[:]))
                    ops.append(lambda t3=t3, vv=vv:
                               nc.vector.tensor_scalar_add(t3[:], vv[:], float(C1)))
                    ops.append(lambda den=den, t3=t3, Q=Q:
                               nc.vector.tensor_mul(den[:], t3[:], Q[:]))
                else:
                    ops.append(lambda P=P, Dv=Dv, u=u:
                               nc.vector.scalar_tensor_tensor(
                                   P[:], Dv, float(C2), u[:],
                                   op0=Alu.add, op1=Alu.subtract))
                    ops.append(lambda num=num, u=u, P=P:
                               nc.vector.scalar_tensor_tensor(
                                   num[:], u[:], float(C1), P[:],
                                   op0=Alu.add, op1=Alu.mult))
                    ops.append(lambda Q=Q, Sv=Sv, vv=vv:
                               nc.vector.scalar_tensor_tensor(
                                   Q[:], Sv, float(C2), vv[:],
                                   op0=Alu.add, op1=Alu.subtract))
                    ops.append(lambda den=den, vv=vv, Q=Q:
                               nc.vector.scalar_tensor_tensor(
                                   den[:], vv[:], float(C1), Q[:],
                                   op0=Alu.add, op1=Alu.mult))
                ops.append(lambda rec=rec, den=den:
                           nc.vector._custom_dve(
                               RECIPROCAL_APPROX_FAST, out=rec[:], in0=den[:],
                               s0=RCST["s0"], s1=RCST["s1"], imm2=RCST["imm2"]))
                if STT_ACCUM:
                    # fused final multiply + per-partition reduce via the
                    # (HW-validated) scalar_tensor_tensor accum_out path
                    ops.append(lambda sout=sout, num=num, rec=rec, slot=slot:
                               nc.vector.scalar_tensor_tensor(
                                   sout[:], num[:], 1.0, rec[:],
                                   op0=Alu.mult, op1=Alu.mult,
                                   accum_out=sums[:, slot:slot + 1]))
                elif NO_TTR:
                    ops.append(lambda sout=sout, num=num, rec=rec:
                               nc.vector.tensor_mul(sout[:], num[:], rec[:]))
                    ops.append(lambda sout=sout, slot=slot:
                               nc.vector.tensor_reduce(
                                   sums[:, slot:slot + 1], sout[:],
                                   axis=mybir.AxisListType.X, op=Alu.add))
                else:
                    ops.append(lambda sout=sout, num=num, rec=rec, slot=slot:
                               nc.vector.tensor_tensor_reduce(
                                   sout[:], num[:], rec[:], 1.0, 0.0,
                                   op0=Alu.mult, op1=Alu.add,
                                   accum_out=sums[:, slot:slot + 1]))
            return ops

        pending = []
        for c in range(C):
            st = spool.tile([128, 4, NK, 2, 128], dt.float16, tag="st")
            prev = None
            for k in range(NK):
                # drain interleaved chain ops first so they never sit behind
                # a PSUM-waiting op in the strict-FIFO engine queues
                for _ in range(3):
                    if pending:
                        pending.pop(0)()
                pa = pass_a(c, k)
                v = bridge(pa)
                if prev is not None:
                    stage(st, k - 1, pass_b(prev))
                    if k == 5:
                        # k tiles 0..3 fully staged: queue first half-channel
                        pending.extend(chain_ops(st, c, 0))
                prev = v
            stage(st, NK - 1, pass_b(prev))
            pending.extend(chain_ops(st, c, 1))
        for op in pending:
            op()

        nc.sync.dma_start(osum[:], sums[:])
    if not nc.is_finalized():
        nc.finalize()
    return nc


def kernel(input, target):
    global last_exec_time_ns, last_results
    from concourse.bass_utils import run_bass_kernel_spmd

    x = np.asarray(input, dtype=np.float32).astype(BF16).astype(np.float32)
    y = np.asarray(target, dtype=np.float32).astype(BF16).astype(np.float32)
    fa = (x + y).astype(BF16)
    fb = (x - y).astype(BF16)
    fm = (2.0 * x * y).astype(BF16)
    fs = (x * x + y * y).astype(BF16)
    wa, wb, w00, w11 = _build_weights()
    wcat = np.ascontiguousarray(np.concatenate([wa, wb, w00, w11], axis=1))

    nc = _build_program()

    in_maps = []
    for core in range(NCORES):
        b, q = core // 4, core % 4
        in_maps.append({
            "fa": _build_slab(fa, b, q),
            "fb": _build_slab(fb, b, q),
            "fm": _build_slab(fm, b, q),
            "fs": _build_slab(fs, b, q),
            "wcat": wcat,
        })

    trace = bool(os.environ.get("SSIM_TRACE"))
    res = run_bass_kernel_spmd(nc, in_maps, list(range(NCORES)), trace=trace)
    last_exec_time_ns = res.exec_time_ns
    last_results = res

    total = np.float64(0.0)
    for r in res.results:
        total += np.asarray(r["osum"], dtype=np.float64).sum()
    n = B * C * T * H * W
    return np.asarray(1.0 - total / n, dtype=np.float32)


# revision 21
# speedup vs baseline: 1.1908x; 1.1908x over previous
"""SSIM3D loss kernel for 8 Trainium2 NeuronCores (v2).

Strategy (hardcoded for inputs [2, 3, 16, 256, 256] fp32):
  - Shard across 8 cores as (batch 2) x (H quarter 4). Each core: C=3,
    T=16, 64 output H rows (+3-row halos), W=256.
  - 4 conv fields: a=x+y, b=x-y, m=2xy, s=x^2+y^2 (all zero in padded
    regions, matching the reference's zero-padded 'same' conv). With
    A1=conv(a), B1=conv(b), D=conv(m), S=conv(s):
      u = (A1^2-B1^2)/2 = 2*mu1*mu2      v = (A1^2+B1^2)/2 = mu1^2+mu2^2
      num = (u+C1)*((D+C2)-u)            den = (v+C1)*((S+C2)-v)
      ssim = num/den
  - Pass A (PE, data-as-lhsT): fused H+T 7-tap conv as banded matmuls,
    partitions packed (h_sub=8, t=16); output transposed to [w, ht].
    H halos via two-matmul PSUM accumulation (wa from j=k, wb from j=k+1).
  - Bridge pa->SBUF bf16 split: DVE copies bank0 half, ACT copies bank1.
  - Pass B (PE, weights-stationary): W 7-tap conv per 128-col w chunk,
    one N=512 matmul per chunk; chunk-boundary taps dropped with
    renormalized truncated windows (golden-sim validated, ~2.7e-4).
  - Staging: ACT Square(sqrt(.5)*x) writes aa/bb straight from PSUM;
    DVE copies D/S from PSUM. Both land in one fp16 stage tile per c.
  - Pointwise chain per half-channel (FD=1024) with fused DVE ops:
    u,v (TT), P/num/Q/den (scalar_tensor_tensor), custom fast reciprocal,
    and tensor_tensor_reduce for the final multiply + partition reduction.
    Chain ops of channel c interleave into channel c+1's k-loop.
  - Host sums the per-core accumulators: loss = 1 - total/N.
"""
import os
import numpy as np
import ml_dtypes

BF16 = ml_dtypes.bfloat16
F16 = np.float16

B, C, T, H, W = 2, 3, 16, 256, 256
WS, SIGMA, PAD = 7, 1.5, 3
C1, C2 = np.float32(1e-4), np.float32(9e-4)
NCORES = 8
HQ = H // 4          # 64 output rows per core
NJ = 9               # input h tiles of 8 rows covering [-3, 69)
NK = 8               # output h tiles of 8 rows covering [0, 64)
FREE = NJ * W        # 2304
NACC = 6             # 3 channels x 2 half-channel groups

last_exec_time_ns = None
last_results = None


def _comp_round(weights):
    """Round to bf16 greedily (largest magnitude first), keeping the
    cumulative rounding error near zero."""
    w = np.asarray(weights, dtype=np.float64).ravel()

    def neighbors(v):
        b = np.float64(np.float32(v).astype(BF16).astype(np.float32))
        cands = {b}
        u = int(np.array(b, dtype=BF16).view(np.uint16))
        for d in (-1, 1):
            cands.add(np.float64(np.uint16((u + d) & 0xFFFF).view(BF16).astype(np.float32)))
        return cands

    order = np.argsort(-np.abs(w))
    out = np.empty_like(w)
    errsum = 0.0
    for i in order:
        best = min(neighbors(w[i]), key=lambda cnd: abs(errsum + (cnd - w[i])))
        out[i] = best
        errsum += best - w[i]
    return out.reshape(np.shape(weights)).astype(np.float32)


def _gaussian():
    coords = np.arange(WS, dtype=np.float64) - PAD
    g = np.exp(-(coords ** 2) / (2.0 * SIGMA ** 2))
    return g / g.sum()


def _build_weights():
    """wa, wb: banded fused H+T conv [128,128].
    W00, W11: per-chunk 1-D W conv [128,128] with renormalized truncated
    windows at the chunk boundary (image edges keep zero-pad truncation)."""
    g = _gaussian()
    wht = _comp_round(np.outer(g, g))

    wa = np.zeros((128, 128), np.float32)
    wb = np.zeros((128, 128), np.float32)
    for i in range(8):
        for o in range(8):
            dh = i - o - 3
            if -3 <= dh <= 3:
                for ti in range(16):
                    for to in range(16):
                        dt_ = ti - to
                        if -3 <= dt_ <= 3:
                            wa[i * 16 + ti, o * 16 + to] = wht[dh + 3, dt_ + 3]
            dh = i + 5 - o
            if -3 <= dh <= 3:
                for ti in range(16):
                    for to in range(16):
                        dt_ = ti - to
                        if -3 <= dt_ <= 3:
                            wb[i * 16 + ti, o * 16 + to] = wht[dh + 3, dt_ + 3]

    gw = _comp_round(g).astype(np.float64)
    Wm = [np.zeros((128, 128), np.float32) for _ in range(2)]
    for m in range(2):
        base = m * 128
        for o in range(128):
            og = base + o
            true_taps = [d for d in range(-3, 4) if 0 <= og + d < W]
            pres = [d for d in true_taps if 0 <= o + d < 128]
            scale = sum(gw[d + 3] for d in true_taps) / sum(gw[d + 3] for d in pres)
            for d in pres:
                Wm[m][o + d, o] = np.float32(gw[d + 3] * scale)
    return (wa.astype(BF16), wb.astype(BF16),
            Wm[0].astype(BF16), Wm[1].astype(BF16))


def _build_slab(x_bf, b, q):
    """Per-core input slab [3, 128, 2304] bf16: partition = h_sub*16+t,
    free = j*256+w; local h = 8j - 3 + h_sub relative to row 64q."""
    pad = np.zeros((C, T, NJ * 8, W), dtype=BF16)
    lo, hi = HQ * q - 3, HQ * q + 69
    s_lo, s_hi = max(0, lo), min(H, hi)
    pad[:, :, (s_lo - lo):(s_hi - lo), :] = x_bf[b, :, :, s_lo:s_hi, :]
    arr = pad.reshape(C, T, NJ, 8, W).transpose(0, 3, 1, 2, 4)
    return np.ascontiguousarray(arr.reshape(C, 128, FREE))


def _build_program():
    import concourse.bass as bass
    import concourse.mybir as mybir
    from concourse import bacc, tile
    from concourse.dve_ops import RECIP_APPROX_FAST_CONSTS, RECIPROCAL_APPROX_FAST
    from contextlib import ExitStack

    dt = mybir.dt
    Alu = mybir.AluOpType
    Act = mybir.ActivationFunctionType
    SQ5 = float(np.sqrt(0.5))
    RCST = RECIP_APPROX_FAST_CONSTS

    nc = bacc.Bacc()
    fin = [nc.dram_tensor(nm, [C, 128, FREE], dt.bfloat16, kind="ExternalInput")
           for nm in ("fa", "fb", "fm", "fs")]
    wdr = nc.dram_tensor("wcat", [128, 512], dt.bfloat16, kind="ExternalInput")
    osum = nc.dram_tensor("osum", [128, NACC], dt.float32, kind="ExternalOutput")

    with tile.TileContext(nc) as tc, ExitStack() as ctx:
        wpool = ctx.enter_context(tc.tile_pool(name="w", bufs=1))
        fpool = ctx.enter_context(tc.tile_pool(name="f", bufs=3))
        vpool = ctx.enter_context(tc.tile_pool(name="v", bufs=3))
        spool = ctx.enter_context(tc.tile_pool(name="st", bufs=2))
        ppool = ctx.enter_context(tc.tile_pool(name="pt", bufs=2))
        psA = ctx.enter_context(tc.tile_pool(name="psA", bufs=2, space="PSUM"))
        psB = ctx.enter_context(tc.tile_pool(name="psB", bufs=2, space="PSUM"))

        # weights: one DMA into staging, one DVE copy bridge so matmuls wait
        # on a single engine semaphore instead of DMA-queue semaphores
        wstg = wpool.tile([128, 512], dt.bfloat16, name="wsg", tag="wsg")
        nc.sync.dma_start(wstg[:], wdr[:])
        wcat = wpool.tile([128, 512], dt.bfloat16)
        nc.vector.tensor_copy(wcat[:], wstg[:])
        wa = wcat[:, 0:128]
        wb = wcat[:, 128:256]
        w00 = wcat[:, 256:384]
        w11 = wcat[:, 384:512]

        sums = wpool.tile([128, NACC], dt.float32)
        c2t = wpool.tile([128, 1], dt.float32)
        nc.gpsimd.memset(c2t[:], float(C2))

        HALF = 5 * 256  # j tiles 0..4 cover pass A for k tiles 0..3
        fields_by_c = []
        for c in range(C):
            ftiles = []
            for i, nm in enumerate(("a", "b", "m", "s")):
                ft = fpool.tile([128, FREE], dt.bfloat16, tag=nm)
                ftiles.append(ft)
            for i in range(4):
                nc.sync.dma_start(ftiles[i][:, 0:HALF], fin[i][c][:, 0:HALF])
            for i in range(4):
                nc.sync.dma_start(ftiles[i][:, HALF:FREE], fin[i][c][:, HALF:FREE])
            fields_by_c.append(tuple(ftiles))

        def pass_a(c, k):
            """8 MMs -> pa [128, 2, 4, 128] (wc, fi, ht)."""
            pa = psA.tile([128, 2, 4, 128], dt.float32, tag="pa")
            fields = fields_by_c[c]
            for wc in range(2):
                for fi in range(4):
                    j0 = k * 256 + wc * 128
                    j1 = (k + 1) * 256 + wc * 128
                    f = fields[fi]
                    nc.tensor.matmul(pa[:, wc, fi], f[:, j0:j0 + 128],
                                     wa, start=True, stop=False)
                    nc.tensor.matmul(pa[:, wc, fi], f[:, j1:j1 + 128],
                                     wb, start=False, stop=True)
            return pa

        NO_TTR = bool(int(os.environ.get("SSIM_NO_TTR", "1")))
        STT_ACCUM = bool(int(os.environ.get("SSIM_STT_ACCUM", "1")))
        NO_STT = bool(int(os.environ.get("SSIM_NO_STT", "0")))
        NO_ACTCOPY = bool(int(os.environ.get("SSIM_NO_ACTCOPY", "0")))
        RANK4 = bool(int(os.environ.get("SSIM_RANK4", "1")))
        NO_GPSIMD = bool(int(os.environ.get("SSIM_NO_GPSIMD", "0")))

        def bridge(pa):
            """pa PSUM -> v SBUF bf16; DVE takes bank pair 0, ACT bank pair 1."""
            v = vpool.tile([128, 2, 4, 128], dt.bfloat16, tag="v")
            nc.vector.tensor_copy(v[:, 0], pa[:, 0])
            if NO_ACTCOPY:
                nc.vector.tensor_copy(v[:, 1], pa[:, 1])
            else:
                nc.scalar.copy(v[:, 1], pa[:, 1])
            return v

        def pass_b(v):
            """2 N=512 MMs -> pb [128, 2, 4, 128] (m, fi, ht), partition=w'."""
            pb = psB.tile([128, 2, 4, 128], dt.float32, tag="pb")
            nc.tensor.matmul(pb[:, 0], w00, v[:, 0], start=True, stop=True)
            nc.tensor.matmul(pb[:, 1], w11, v[:, 1], start=True, stop=True)
            return pb

        def stage(st, k, pb):
            """aa/bb via ACT Square from PSUM; D/S via ACT Identity with the
            C2 bias folded in (so the chain's P/Q become plain tensor ops).
            st layout is quantity-major [128, q, k, m, ht] -> chain views are
            dense rank-2.  pb free dims are (m, q, ht); st views are (q, m,
            ht), so the pb access patterns transpose m and q to match."""
            if RANK4:
                pq = pb[:].rearrange("p m q n -> p q m n")
                nc.scalar.activation(st[:, 0:2, k, :, :], pq[:, 0:2, :, :],
                                     Act.Square, scale=SQ5)
                nc.scalar.activation(st[:, 2:4, k, :, :], pq[:, 2:4, :, :],
                                     Act.Identity, bias=c2t[:])
            else:
                for m in range(2):
                    nc.scalar.activation(st[:, 0:2, k, m, :], pb[:, m, 0:2, :],
                                         Act.Square, scale=SQ5)
                    nc.scalar.activation(st[:, 2:4, k, m, :], pb[:, m, 2:4, :],
                                         Act.Identity, bias=c2t[:])

        def chain_ops(st, c, g):
            """8 lambdas: pointwise for half-channel group g of channel c."""
            ops = []
            if True:
                k0, k1 = g * 4, g * 4 + 4
                mrg = lambda q: st[:, q, k0:k1, :, :].rearrange(
                    "p a b n -> p (a b n)")
                aa, bb, Dv, Sv = mrg(0), mrg(1), mrg(2), mrg(3)
                shape = [128, 1024]
                u = ppool.tile(shape, dt.bfloat16, tag="u")
                vv = ppool.tile(shape, dt.bfloat16, tag="vv")
                L = ppool.tile(shape, dt.bfloat16, tag="L")
                M = ppool.tile(shape, dt.bfloat16, tag="M")
                P = ppool.tile(shape, dt.bfloat16, tag="P")
                num = ppool.tile(shape, dt.bfloat16, tag="num")
                Q = ppool.tile(shape, dt.bfloat16, tag="Q")
                den = ppool.tile(shape, dt.float32, tag="den")
                rec = ppool.tile(shape, dt.bfloat16, tag="rec")
                sout = ppool.tile(shape, dt.bfloat16, tag="sout")
                slot = c * 2 + g
                teng = nc.vector if NO_GPSIMD else nc.gpsimd
                ops.append(lambda u=u, aa=aa, bb=bb, teng=teng:
                           teng.tensor_sub(u[:], aa, bb))
                ops.append(lambda vv=vv, aa=aa, bb=bb, teng=teng:
                           teng.tensor_add(vv[:], aa, bb))
                if False:
                    t0 = ppool.tile(shape, dt.float16, tag="t0")
                    t1 = ppool.tile(shape, dt.float16, tag="t1")
                    ops.append(lambda t0=t0, Dv=Dv:
                               nc.vector.tensor_scalar_add(t0[:], Dv, float(C2)))
                    ops.append(lambda P=P, t0=t0, u=u:
                               nc.vector.tensor_sub(P[:], t0[:], u[:]))
                    ops.append(lambda t1=t1, u=u:
                               nc.vector.tensor_scalar_add(t1[:], u[:], float(C1)))
                    ops.append(lambda num=num, t1=t1, P=P:
                               nc.vector.tensor_mul(num[:], t1[:], P[:]))
                    t2 = ppool.tile(shape, dt.float16, tag="t2")
                    t3 = ppool.tile(shape, dt.float16, tag="t3")
                    ops.append(lambda t2=t2, Sv=Sv:
                               nc.vector.tensor_scalar_add(t2[:], Sv, float(C2)))
                    ops.append(lambda Q=Q, t2=t2, vv=vv:
                               nc.vector.tensor_sub(Q[:], t2[:], vv[:]))
                    ops.append(lambda t3=t3, vv=vv:
                               nc.vector.tensor_scalar_add(t3[:], vv[:], float(C1)))
                    ops.append(lambda den=den, t3=t3, Q=Q:
                               nc.vector.tensor_mul(den[:], t3[:], Q[:]))
                else:
                    # staged Dv/Sv already carry +C2: P = Dv' - u, Q = Sv' - vv
                    ops.append(lambda P=P, Dv=Dv, u=u:
                               nc.vector.tensor_sub(P[:], Dv, u[:]))
                    ops.append(lambda L=L, u=u:
                               nc.vector.tensor_scalar_add(L[:], u[:], float(C1)))
                    ops.append(lambda num=num, L=L, P=P:
                               nc.vector.tensor_mul(num[:], L[:], P[:]))
                    ops.append(lambda Q=Q, Sv=Sv, vv=vv:
                               nc.vector.tensor_sub(Q[:], Sv, vv[:]))
                    ops.append(lambda M=M, vv=vv:
                               nc.vector.tensor_scalar_add(M[:], vv[:], float(C1)))
                    ops.append(lambda den=den, M=M, Q=Q:
                               nc.vector.tensor_mul(den[:], M[:], Q[:]))
                ops.append(lambda rec=rec, den=den:
                           nc.vector._custom_dve(
                               RECIPROCAL_APPROX_FAST, out=rec[:], in0=den[:],
                               s0=RCST["s0"], s1=RCST["s1"], imm2=RCST["imm2"]))
                if STT_ACCUM:
                    # fused final multiply + per-partition reduce via the
                    # (HW-validated) scalar_tensor_tensor accum_out path
                    ops.append(lambda sout=sout, num=num, rec=rec, slot=slot:
                               nc.vector.scalar_tensor_tensor(
                                   sout[:], num[:], 1.0, rec[:],
                                   op0=Alu.mult, op1=Alu.mult,
                                   accum_out=sums[:, slot:slot + 1]))
                elif NO_TTR:
                    ops.append(lambda sout=sout, num=num, rec=rec:
                               nc.vector.tensor_mul(sout[:], num[:], rec[:]))
                    ops.append(lambda sout=sout, slot=slot:
                               nc.vector.tensor_reduce(
                                   sums[:, slot:slot + 1], sout[:],
                                   axis=mybir.AxisListType.X, op=Alu.add))
                else:
                    ops.append(lambda sout=sout, num=num, rec=rec, slot=slot:
                               nc.vector.tensor_tensor_reduce(
                                   sout[:], num[:], rec[:], 1.0, 0.0,
                                   op0=Alu.mult, op1=Alu.add,
                                   accum_out=sums[:, slot:slot + 1]))
            return ops

        pending = []
        for c in range(C):
            st = spool.tile([128, 4, NK, 2, 128], dt.bfloat16, tag="st")
            prev = None
            for k in range(NK):
                # drain interleaved chain ops first so they never sit behind
                # a PSUM-waiting op in the strict-FIFO engine queues
                for _ in range(3):
                    if pending:
                        pending.pop(0)()
                pa = pass_a(c, k)
                v = bridge(pa)
                if prev is not None:
                    stage(st, k - 1, pass_b(prev))
                    if k == 5:
                        # k tiles 0..3 fully staged: queue first half-channel
                        pending.extend(chain_ops(st, c, 0))
                prev = v
            stage(st, NK - 1, pass_b(prev))
            pending.extend(chain_ops(st, c, 1))
        for op in pending:
            op()

        nc.sync.dma_start(osum[:], sums[:])
    if not nc.is_finalized():
        nc.finalize()
    return nc


def kernel(input, target):
    global last_exec_time_ns, last_results
    from concourse.bass_utils import run_bass_kernel_spmd

    x = np.asarray(input, dtype=np.float32).astype(BF16).astype(np.float32)
    y = np.asarray(target, dtype=np.float32).astype(BF16).astype(np.float32)
    fa = (x + y).astype(BF16)
    fb = (x - y).astype(BF16)
    fm = (2.0 * x * y).astype(BF16)
    fs = (x * x + y * y).astype(BF16)
    wa, wb, w00, w11 = _build_weights()
    wcat = np.ascontiguousarray(np.concatenate([wa, wb, w00, w11], axis=1))

    nc = _build_program()

    in_maps = []
    for core in range(NCORES):
        b, q = core // 4, core % 4
        in_maps.append({
            "fa": _build_slab(fa, b, q),
            "fb": _build_slab(fb, b, q),
            "fm": _build_slab(fm, b, q),
            "fs": _build_slab(fs, b, q),
            "wcat": wcat,
        })

    trace = bool(os.environ.get("SSIM_TRACE"))
    res = run_bass_kernel_spmd(nc, in_maps, list(range(NCORES)), trace=trace)
    last_exec_time_ns = res.exec_time_ns
    last_results = res

    total = np.float64(0.0)
    for r in res.results:
        total += np.asarray(r["osum"], dtype=np.float64).sum()
    n = B * C * T * H * W
    return np.asarray(1.0 - total / n, dtype=np.float32)
